# revision 4
# baseline (speedup 1.0000x reference)
"""AttnDecoderRNN step on 8 Trainium2 NeuronCores (Bass/Tile SPMD).

Sharding: attention is sharded over encoder L / decoder T across cores
(full batch per core); the output projection is sharded column-wise over
the vocabulary (6250 columns per core). Three collectives: AllGather of
exp(score) parts, AllReduce of unnormalized context partials, AllReduce
of the per-row exp-sum for log_softmax. The pointer-scatter is done with
indirect DMAs (compaction gather + scatter-add with CCE accumulate).
"""
import numpy as np

import concourse.bass as bass
import concourse.mybir as mybir
import concourse.tile as tile
from concourse.tile import add_dep_helper
from concourse.bass_utils import run_bass_kernel_spmd

# ---------------- problem constants (hardcoded) ----------------
NC = 8
B = 128
E = 256
H = 512
L = 400
T = 100
V = 50000
VS = V // NC          # 6250 vocab cols per core
LS = L // NC          # 50 encoder rows per core
TP = 104              # decoder padded to 8*13
TS = TP // NC         # 13 decoder rows per core
KH = H // 128         # 4
NCOL = VS + 2         # big matmul width (+ pointer column, + pad col for even-N fp32r)
NDUMP = 128           # dump zone appended to the scatter target

DT = mybir.dt.float32
F32R = mybir.dt.float32r
I32 = mybir.dt.int32
AOP = mybir.AluOpType
AFT = mybir.ActivationFunctionType

_cache = {}


def _split_sync_waits(nc, max_waits=1):
    """walrus build only allows 1 sync-wait per engine instruction: move
    excess waits onto same-engine NOP carriers placed just before."""
    nop_id = [0]

    def mk_nop(engine, waits):
        nop_id[0] += 1
        return mybir.InstNoOp(
            name=f"waitnop-{nop_id[0]}",
            engine=engine,
            ins=[],
            outs=[],
            sync_info=mybir.SyncInfo(on_wait=list(waits), on_update=[]),
        )

    for func in nc.m.functions:
        for bb in func.blocks:
            insts = bb.instructions
            out = []
            changed = False
            for ins in insts:
                si = ins.sync_info
                if si is not None and si.on_wait and len(si.on_wait) > max_waits:
                    nm = type(ins).__name__
                    if nm not in ("InstTensorLoad", "InstTensorSave",
                                  "InstDMATrigger", "InstTriggeredCopy"):
                        waits = list(si.on_wait)
                        while len(waits) > max_waits:
                            chunk, waits = waits[:max_waits], waits[max_waits:]
                            out.append(mk_nop(ins.engine, chunk))
                        ins.sync_info = mybir.SyncInfo(
                            on_wait=waits, on_update=si.on_update
                        )
                        changed = True
                out.append(ins)
            if changed:
                insts[:] = out


def _build_program(n_pads):
    """Build the SPMD Bass program. n_pads: tuple of padded entry counts
    (multiple of 128) for each scatter round."""
    nc = bass.Bass()

    # ---- I/O ----
    xT = nc.dram_tensor("xT", [E + 1, B], F32R, kind="ExternalInput")
    h0T = nc.dram_tensor("h0T", [H + 1, B], F32R, kind="ExternalInput")
    h0 = nc.dram_tensor("h0", [B, H], DT, kind="ExternalInput")
    w_ih = nc.dram_tensor("w_ih", [E + 1, 3 * H], F32R, kind="ExternalInput")
    w_hh = nc.dram_tensor("w_hh", [H + 1, 3 * H], F32R, kind="ExternalInput")
    w_attn = nc.dram_tensor("w_attn", [H + 1, 2 * H], F32R, kind="ExternalInput")
    onesr = nc.dram_tensor("onesr", [1, B], F32R, kind="ExternalInput")
    enc = nc.dram_tensor("enc", [LS, B, H], DT, kind="ExternalInput")
    dec = nc.dram_tensor("dec", [TS, B, H], DT, kind="ExternalInput")
    mask_d = nc.dram_tensor("mask_d", [B, TS], DT, kind="ExternalInput")
    w_big = nc.dram_tensor("w_big", [12 * 128 + 1, NCOL], F32R, kind="ExternalInput")
    goffs_in = [
        nc.dram_tensor(f"goffs{r}", [128, n // 128], I32, kind="ExternalInput")
        for r, n in enumerate(n_pads)
    ]
    soffs_in = [
        nc.dram_tensor(f"soffs{r}", [128, n // 128], I32, kind="ExternalInput")
        for r, n in enumerate(n_pads)
    ]

    y = nc.dram_tensor("y", [B, VS], DT, kind="ExternalOutput")
    h_out = nc.dram_tensor("h_out", [B, H], DT, kind="ExternalOutput")

    mask_dram = nc.dram_tensor("mask_scratch", [B * VS + NDUMP], DT)
    alpha_dram = nc.dram_tensor("alpha_scratch", [B * L], DT)

    # big-matmul passes over the vocab shard: pointer-column pass first
    passes = [(3072, NCOL), (0, 3072)]

    with tile.TileContext(nc) as tc:
        with (
            tc.tile_pool(name="persist", bufs=1) as pp,
            tc.tile_pool(name="mm", bufs=1) as mp,
            tc.tile_pool(name="psT", bufs=1, space="PSUM") as psT,
            tc.tile_pool(name="dram", bufs=1, space="DRAM") as dp,
        ):
            # ---------- kick off the W_big stream immediately ----------
            rhs_tiles = {}
            for pi, (c0, c1) in enumerate(passes):
                w = c1 - c0
                for k in range(12):
                    rt = mp.tile([128, w], F32R, name=f"rhs{pi}_{k}", tag="rhs", bufs=3)
                    nc.sync.dma_start(rt[:], w_big[k * 128:(k + 1) * 128, c0:c1])
                    rhs_tiles[(pi, k)] = rt
                rb = mp.tile([1, w], F32R, name=f"rhsb{pi}", tag="rhsb", bufs=1)
                nc.sync.dma_start(rb[:], w_big[12 * 128:12 * 128 + 1, c0:c1])
                rhs_tiles[(pi, "bias")] = rb

            # ---------- identity for PE transpose ----------
            ii = pp.tile([128, 128], DT)
            nc.gpsimd.iota(ii[:], [[1, 128]], channel_multiplier=-1,
                           allow_small_or_imprecise_dtypes=True)
            ident = pp.tile([128, 128], DT)
            nc.vector.tensor_scalar(ident[:], ii[:], 0.0, None, AOP.is_equal)

            ones_r = pp.tile([1, B], F32R)
            nc.sync.dma_start(ones_r[:], onesr[:])

            # ---------- zero the mask scratch early ----------
            with tc.tile_pool(name="zero", bufs=1) as zp:
                zt = zp.tile([B, 2048], DT)
                nc.vector.memset(zt[:], 0.0)
                m2d = mask_dram[: B * VS].rearrange("(b v) -> b v", v=VS)
                mz_wrs = []
                for c in range(0, VS, 2048):
                    cw = min(2048, VS - c)
                    mz_wrs.append(nc.sync.dma_start(m2d[:, c:c + cw], zt[:, :cw]))
                mz_wrs.append(nc.sync.dma_start(mask_dram[B * VS:][None, :], zt[:1, :NDUMP]))

            # ---------- GRU ----------
            xT_t = pp.tile([128, 2, B], F32R)
            for k in range(2):
                nc.sync.dma_start(xT_t[:, k, :], xT[k * 128:(k + 1) * 128, :])
            x_ones = pp.tile([1, B], F32R)
            nc.sync.dma_start(x_ones[:], xT[E:E + 1, :])
            h0T_t = pp.tile([128, 4, B], F32R)
            for k in range(4):
                nc.sync.dma_start(h0T_t[:, k, :], h0T[k * 128:(k + 1) * 128, :])
            h0_ones = pp.tile([1, B], F32R)
            nc.sync.dma_start(h0_ones[:], h0T[H:H + 1, :])
            h0_sb = pp.tile([B, H], DT)
            nc.sync.dma_start(h0_sb[:], h0[:])

            h_sb = pp.tile([B, H], DT)
            hT = pp.tile([128, KH, B], F32R, name="hT")
            te_sb = pp.tile([B, 2 * H], DT)

            with tc.tile_pool(name="gru", bufs=1) as gp, \
                 tc.tile_pool(name="psA", bufs=1, space="PSUM") as psA:
                def wchunk(src, k, width):
                    t = gp.tile([128, width], F32R, name=f"wc{k}", tag="wc", bufs=2)
                    nc.sync.dma_start(t[:], src)
                    return t

                def wbias(src, width):
                    t = gp.tile([1, width], F32R, name="wb", tag="wb", bufs=2)
                    nc.sync.dma_start(t[:], src)
                    return t

                # gh = h0_aug @ w_hh  -> PSUM (3 banks)
                gh_ps = [psA.tile([B, 512], DT, name=f"ghps{n}", tag="psa", bufs=3)
                         for n in range(3)]
                for k in range(4):
                    wt = wchunk(w_hh[k * 128:(k + 1) * 128, :], k, 3 * H)
                    for n in range(3):
                        nc.tensor.matmul(gh_ps[n][:], h0T_t[:, k, :],
                                         wt[:, n * 512:(n + 1) * 512],
                                         start=(k == 0), stop=False)
                wb = wbias(w_hh[H:H + 1, :], 3 * H)
                for n in range(3):
                    nc.tensor.matmul(gh_ps[n][:], h0_ones[:], wb[:, n * 512:(n + 1) * 512],
                                     start=False, stop=True)
                gh_sb = gp.tile([B, 3 * H], DT)
                for n in range(3):
                    nc.scalar.copy(gh_sb[:, n * 512:(n + 1) * 512], gh_ps[n][:])

                gi_ps = [psA.tile([B, 512], DT, name=f"gips{n}", tag="psa", bufs=3)
                         for n in range(3)]
                for k in range(2):
                    wt = wchunk(w_ih[k * 128:(k + 1) * 128, :], 10 + k, 3 * H)
                    for n in range(3):
                        nc.tensor.matmul(gi_ps[n][:], xT_t[:, k, :],
                                         wt[:, n * 512:(n + 1) * 512],
                                         start=(k == 0), stop=False)
                wb = wbias(w_ih[E:E + 1, :], 3 * H)
                for n in range(3):
                    nc.tensor.matmul(gi_ps[n][:], x_ones[:], wb[:, n * 512:(n + 1) * 512],
                                     start=False, stop=True)

                # gates
                rz = gp.tile([B, 2 * H], DT)
                nc.vector.tensor_tensor(rz[:, :512], gi_ps[0][:], gh_sb[:, :512], op=AOP.add)
                nc.vector.tensor_tensor(rz[:, 512:], gi_ps[1][:], gh_sb[:, 512:1024], op=AOP.add)
                nc.scalar.activation(rz[:], rz[:], AFT.Sigmoid)
                nt = gp.tile([B, H], DT)
                nc.vector.tensor_tensor(nt[:], rz[:, :512], gh_sb[:, 1024:], op=AOP.mult)
                nc.vector.tensor_tensor(nt[:], nt[:], gi_ps[2][:], op=AOP.add)
                nc.scalar.activation(nt[:], nt[:], AFT.Tanh)
                nc.vector.tensor_tensor(h_sb[:], h0_sb[:], nt[:], op=AOP.subtract)
                nc.vector.tensor_tensor(h_sb[:], h_sb[:], rz[:, 512:], op=AOP.mult)
                nc.vector.tensor_tensor(h_sb[:], h_sb[:], nt[:], op=AOP.add)
                nc.sync.dma_start(h_out[:], h_sb[:])

                # hT via PE transpose
                tp_ps = psA.tile([128, 128], DT, name="tp", tag="tpps", bufs=1)
                for k in range(KH):
                    nc.tensor.transpose(tp_ps[:], h_sb[:, k * 128:(k + 1) * 128], ident[:])
                    nc.scalar.copy(hT[:, k, :], tp_ps[:])

                # te|td = h_aug @ w_attn
                tt_ps = [psA.tile([B, 512], DT, name=f"ttps{n}", tag="psa", bufs=3)
                         for n in range(2)]
                for k in range(4):
                    wt = wchunk(w_attn[k * 128:(k + 1) * 128, :], 20 + k, 2 * H)
                    for n in range(2):
                        nc.tensor.matmul(tt_ps[n][:], hT[:, k, :],
                                         wt[:, n * 512:(n + 1) * 512],
                                         start=(k == 0), stop=False)
                wb = wbias(w_attn[H:H + 1, :], 2 * H)
                for n in range(2):
                    nc.tensor.matmul(tt_ps[n][:], ones_r[:], wb[:, n * 512:(n + 1) * 512],
                                     start=False, stop=True)
                for n in range(2):
                    nc.scalar.copy(te_sb[:, n * 512:(n + 1) * 512], tt_ps[n][:])

            # ---------- attention (sharded over L/T), pipelined per row ----------
            payload = pp.tile([B, 64], DT)
            nc.vector.memset(payload[:], 0.0)
            e_cols = pp.tile([B, 64], DT)
            u_acc = pp.tile([B, 2 * H], DT)
            nc.vector.memset(u_acc[:], 0.0)

            with tc.tile_pool(name="attn", bufs=1) as ap:
                scr = ap.tile([B, H], DT, name="scr")
                md = ap.tile([B, TS], DT, name="md")
                nc.sync.dma_start(md[:], mask_d[:])
                for l in range(LS + TS):
                    is_enc = l < LS
                    src = enc[l] if is_enc else dec[l - LS]
                    t = ap.tile([B, H], DT, name=f"row{l}", tag="row", bufs=8)
                    nc.sync.dma_start(t[:], src)
                    tesl = te_sb[:, :512] if is_enc else te_sb[:, 512:]
                    nc.vector.scalar_tensor_tensor(
                        scr[:], tesl, 1.0, t[:],
                        op0=AOP.mult, op1=AOP.mult,
                        accum_out=e_cols[:, l:l + 1])
                    nc.scalar.activation(payload[:, l:l + 1], e_cols[:, l:l + 1], AFT.Exp)
                    if not is_enc:
                        nc.vector.tensor_tensor(
                            payload[:, l:l + 1], payload[:, l:l + 1],
                            md[:, l - LS:l - LS + 1], op=AOP.mult)
                    usl = u_acc[:, :512] if is_enc else u_acc[:, 512:]
                    nc.vector.scalar_tensor_tensor(
                        usl, t[:], payload[:, l:l + 1], usl,
                        op0=AOP.mult, op1=AOP.add)

                # collective 1: AllGather exp parts
                cc1_in = dp.tile([B, 64], DT)
                cc1_out = dp.tile([NC * B, 64], DT)
                nc.sync.dma_start(cc1_in[:], payload[:])
                nc.gpsimd.collective_compute(
                    "AllGather", AOP.bypass,
                    replica_groups=[list(range(NC))],
                    ins=[cc1_in.opt()], outs=[cc1_out.opt()])

                # collective 2: AllReduce context partials
                cc2_in = dp.tile([B, 2 * H], DT)
                cc2_out = dp.tile([B, 2 * H], DT)
                nc.sync.dma_start(cc2_in[:], u_acc[:])
                nc.gpsimd.collective_compute(
                    "AllReduce", AOP.add,
                    replica_groups=[list(range(NC))],
                    ins=[cc2_in.opt()], outs=[cc2_out.opt()])

            # read back collective results
            gath = pp.tile([B, NC, 64], DT)
            nc.sync.dma_start(
                gath[:, :, :],
                cc1_out[:].rearrange("(r p) c -> p r c", p=B))
            u_all = pp.tile([B, 2 * H], DT)
            nc.sync.dma_start(u_all[:], cc2_out[:])

            # ---------- normalize: Z, c_e, c_d, alpha ----------
            zcol = pp.tile([B, 4], DT)
            nc.vector.tensor_reduce(zcol[:, 0:1], gath[:, :, 0:LS], axis=mybir.AxisListType.XY, op=AOP.add)
            nc.vector.tensor_reduce(zcol[:, 1:2], gath[:, :, LS:LS + TS], axis=mybir.AxisListType.XY, op=AOP.add)
            nc.vector.reciprocal(zcol[:, 2:3], zcol[:, 0:1])
            nc.vector.reciprocal(zcol[:, 3:4], zcol[:, 1:2])

            nh = pp.tile([B, 3 * H], DT)  # [c_e | h | c_d]
            nc.vector.tensor_scalar(nh[:, :512], u_all[:, :512], zcol[:, 2:3], None, AOP.mult)
            nc.vector.tensor_copy(nh[:, 512:1024], h_sb[:])
            nc.vector.tensor_scalar(nh[:, 1024:], u_all[:, 512:], zcol[:, 3:4], None, AOP.mult)

            alpha = pp.tile([B, L], DT)
            nc.vector.tensor_scalar(alpha[:], gath[:, :, 0:LS], zcol[:, 2:3], None, AOP.mult)

            # nhT for the big matmul
            nhT = pp.tile([128, 12, B], F32R, name="nhT")
            tp2_ps = psT.tile([128, 128], DT, name="tp2", tag="tp2", bufs=1)
            for k in range(12):
                nc.tensor.transpose(tp2_ps[:], nh[:, k * 128:(k + 1) * 128], ident[:])
                nc.scalar.copy(nhT[:, k, :], tp2_ps[:])

            with tc.tile_pool(name="late", bufs=1) as lp, \
                 tc.tile_pool(name="psB", bufs=1, space="PSUM") as psB:
                logits = lp.tile([B, NCOL], DT)
                # ---------- big matmul, k-outer, psum n-tiles resident ----------
                for pi, (c0, c1) in enumerate(passes):
                    w = c1 - c0
                    ntiles = (w + 511) // 512
                    pst = [psB.tile([B, min(512, w - nt * 512)], DT,
                                    name=f"mmps{pi}_{nt}", tag="mmps", bufs=7)
                           for nt in range(ntiles)]
                    for k in range(12):
                        rt = rhs_tiles[(pi, k)]
                        for nt in range(ntiles):
                            s0 = nt * 512
                            s1 = min(w, s0 + 512)
                            nc.tensor.matmul(pst[nt][:], nhT[:, k, :], rt[:, s0:s1],
                                             start=(k == 0), stop=False)
                    rb = rhs_tiles[(pi, "bias")]
                    for nt in range(ntiles):
                        s0 = nt * 512
                        s1 = min(w, s0 + 512)
                        nc.tensor.matmul(pst[nt][:], ones_r[:], rb[:, s0:s1],
                                         start=False, stop=True)
                        nc.scalar.copy(logits[:, c0 + s0:c0 + s1], pst[nt][:])

                # ---------- pointer prob, alpha_scaled, scatter ----------
                pcol = pp.tile([B, 4], DT)
                nc.vector.tensor_copy(pcol[:, 0:1], logits[:, VS:VS + 1])
                onecol = pp.tile([B, 1], DT)
                nc.vector.memset(onecol[:], 1.0)
                nc.vector.scalar_tensor_tensor(pcol[:, 1:2], pcol[:, 0:1], -1.0, onecol[:],
                                               op0=AOP.mult, op1=AOP.add)

                alpha_s = pp.tile([B, L], DT)
                nc.vector.tensor_scalar(alpha_s[:], alpha[:], pcol[:, 0:1], None, AOP.mult)
                a2d = alpha_dram[:].rearrange("(b l) -> b l", l=L)
                a_wr = nc.sync.dma_start(a2d[:, :], alpha_s[:])

                prev_sc = None
                for r, n_pad in enumerate(n_pads):
                    fc = n_pad // 128
                    go = lp.tile([128, fc], I32, name=f"go{r}", tag="go", bufs=2)
                    nc.sync.dma_start(go[:], goffs_in[r][:])
                    so = lp.tile([128, fc], I32, name=f"so{r}", tag="so", bufs=2)
                    nc.sync.dma_start(so[:], soffs_in[r][:])
                    vals = lp.tile([1, n_pad], DT, name=f"vals{r}", tag="vals", bufs=1)
                    g = nc.gpsimd.indirect_dma_start(
                        out=vals[:, :, None], out_offset=None,
                        in_=alpha_dram[:, None],
                        in_offset=bass.IndirectOffsetOnAxis(ap=go[:], axis=0),
                        bounds_check=B * L - 1, oob_is_err=False)
                    add_dep_helper(g.ins, a_wr.ins, reason="gather after alpha write")
                    sc = nc.gpsimd.indirect_dma_start(
                        out=mask_dram[:, None],
                        out_offset=bass.IndirectOffsetOnAxis(ap=so[:], axis=0),
                        in_=vals[:, :, None], in_offset=None,
                        bounds_check=B * VS + NDUMP - 1, oob_is_err=False,
                        compute_op=AOP.add)
                    for mzw in mz_wrs:
                        add_dep_helper(sc.ins, mzw.ins, reason="scatter after zero")
                    add_dep_helper(sc.ins, g.ins, reason="scatter after gather")
                    if prev_sc is not None:
                        add_dep_helper(sc.ins, prev_sc.ins, reason="round serialization")
                    prev_sc = sc

                # ---------- log-softmax denominator ----------
                esc = lp.tile([B, VS], DT)
                scol = pp.tile([B, 2], DT)
                nc.scalar.activation(esc[:], logits[:, :VS], AFT.Exp,
                                     accum_out=scol[:, 0:1])
                cc3_in = dp.tile([B, 1], DT)
                cc3_out = dp.tile([B, 1], DT)
                nc.sync.dma_start(cc3_in[:], scol[:, 0:1])
                nc.gpsimd.collective_compute(
                    "AllReduce", AOP.add,
                    replica_groups=[list(range(NC))],
                    ins=[cc3_in.opt()], outs=[cc3_out.opt()])
                sall = pp.tile([B, 2], DT)
                nc.sync.dma_start(sall[:, 0:1], cc3_out[:])
                nc.scalar.activation(sall[:, 1:2], sall[:, 0:1], AFT.Ln)

                # ---------- final: (logits - logZ)*(1-p) + mask ----------
                nc.vector.tensor_scalar(logits[:, :VS], logits[:, :VS],
                                        sall[:, 1:2], pcol[:, 1:2],
                                        AOP.subtract, AOP.mult)
                msk = lp.tile([B, VS], DT, name="msk", tag="esc_msk", bufs=1)
                m2d2 = mask_dram[: B * VS].rearrange("(b v) -> b v", v=VS)
                mrd = nc.sync.dma_start(msk[:], m2d2[:, :])
                if prev_sc is not None:
                    add_dep_helper(mrd.ins, prev_sc.ins, reason="mask read after scatter")
                nc.vector.tensor_tensor(logits[:, :VS], logits[:, :VS], msk[:], op=AOP.add)
                nc.sync.dma_start(y[:], logits[:, :VS])

    _split_sync_waits(nc, max_waits=1)
    return nc


def kernel(inputs, hidden, encoder_hidden, encoder_lengths, previous_weights,
           decoder_hidden, encoder_inputs, time_step,
           emb, W_ih, W_hh, b_ih, b_hh, W_ae, b_ae, W_ad, b_ad,
           W_ptr, b_ptr, W_out, b_out):
    f32 = np.float32
    inputs = np.asarray(inputs)
    hidden = np.asarray(hidden, f32)
    encoder_hidden = np.asarray(encoder_hidden, f32)
    previous_weights = np.asarray(previous_weights, f32)
    decoder_hidden = np.asarray(decoder_hidden, f32)
    encoder_inputs = np.asarray(encoder_inputs)
    emb = np.asarray(emb, f32)
    W_ih = np.asarray(W_ih, f32); W_hh = np.asarray(W_hh, f32)
    b_ih = np.asarray(b_ih, f32); b_hh = np.asarray(b_hh, f32)
    W_ae = np.asarray(W_ae, f32); b_ae = np.asarray(b_ae, f32)
    W_ad = np.asarray(W_ad, f32); b_ad = np.asarray(b_ad, f32)
    W_ptr = np.asarray(W_ptr, f32); b_ptr = np.asarray(b_ptr, f32)
    W_out = np.asarray(W_out, f32); b_out = np.asarray(b_out, f32)

    # ---- shared host prep (layout only) ----
    x = emb[np.asarray(inputs[0], np.int64)]                # (B, E) row gather
    xT_aug = np.concatenate([x.T, np.ones((1, B), f32)], 0)  # (E+1, B)
    h0 = hidden[0]                                          # (B, H)
    h0T_aug = np.concatenate([h0.T, np.ones((1, B), f32)], 0)
    w_ih_rhs = np.ascontiguousarray(
        np.concatenate([W_ih.T, b_ih[None, :]], 0))          # (E+1, 3H)
    w_hh_rhs = np.ascontiguousarray(
        np.concatenate([W_hh.T, b_hh[None, :]], 0))          # (H+1, 3H)
    w_attn_rhs = np.ascontiguousarray(np.concatenate([
        np.concatenate([W_ae.T, W_ad.T], 1),
        np.concatenate([b_ae, b_ad])[None, :]], 0))          # (H+1, 2H)
    ones_row = np.ones((1, B), f32)
    dec_pad = np.zeros((TP, B, H), f32)
    dec_pad[:T] = decoder_hidden
    WoutT = W_out.T                                          # (3H, V)
    enc_inT = np.ascontiguousarray(encoder_inputs.T).astype(np.int64)  # (B, L)

    # ---- per-core scatter entry lists (with duplicate rounds) ----
    core_rounds = []  # [core][round] -> list of (gpos, spos)
    max_rounds = 1
    for i in range(NC):
        v0 = i * VS
        rounds = [[]]
        seen = {}
        vv = enc_inT - v0
        inshard = (vv >= 0) & (vv < VS)
        bs, ls = np.nonzero(inshard)
        for b, l in zip(bs.tolist(), ls.tolist()):
            vloc = int(vv[b, l])
            key = b * VS + vloc
            r = seen.get(key, 0)
            seen[key] = r + 1
            while len(rounds) <= r:
                rounds.append([])
            rounds[r].append((b * L + l, key))
        core_rounds.append(rounds)
        max_rounds = max(max_rounds, len(rounds))

    n_pads = []
    for r in range(max_rounds):
        mx = max(len(cr[r]) if r < len(cr) else 0 for cr in core_rounds)
        n_pads.append(max(128, ((mx + 127) // 128) * 128))
    n_pads = tuple(n_pads)

    key = n_pads
    if key not in _cache:
        _cache[key] = _build_program(n_pads)
    nc = _cache[key]

    # ---- per-core input maps ----
    in_maps = []
    for i in range(NC):
        v0 = i * VS
        w_big = np.zeros((12 * 128 + 1, NCOL), f32)
        w_big[:3 * H, :VS] = WoutT[:, v0:v0 + VS]
        w_big[:3 * H, VS] = W_ptr[0]
        w_big[3 * H, :VS] = b_out[v0:v0 + VS]
        w_big[3 * H, VS] = b_ptr[0]

        mask_np = np.zeros((B, TS), f32)
        t_lo = i * TS
        valid = min(TS, max(0, T - t_lo))
        mask_np[:, :valid] = 1.0

        m = {
            "xT": xT_aug, "h0T": h0T_aug, "h0": h0,
            "w_ih": w_ih_rhs, "w_hh": w_hh_rhs, "w_attn": w_attn_rhs,
            "onesr": ones_row,
            "enc": np.ascontiguousarray(encoder_hidden[i * LS:(i + 1) * LS]),
            "dec": np.ascontiguousarray(dec_pad[i * TS:(i + 1) * TS]),
            "mask_d": mask_np,
            "w_big": w_big,
        }
        rounds = core_rounds[i]
        for r, n_pad in enumerate(n_pads):
            entries = rounds[r] if r < len(rounds) else []
            go = np.zeros((128, n_pad // 128), np.int32)
            so = np.full((128, n_pad // 128), B * VS, np.int32)  # pad -> dump
            jj = np.arange(len(entries))
            if len(entries):
                gp = np.array([e[0] for e in entries], np.int32)
                sp = np.array([e[1] for e in entries], np.int32)
                go[jj % 128, jj // 128] = gp
                so[jj % 128, jj // 128] = sp
            m[f"goffs{r}"] = go
            m[f"soffs{r}"] = so
        in_maps.append(m)

    res = run_bass_kernel_spmd(nc, in_maps, list(range(NC)))
    final = np.concatenate([res.results[i]["y"] for i in range(NC)], axis=1)
    h_new = res.results[0]["h_out"][None]
    return final, h_new, previous_weights
